# revision 11
# baseline (speedup 1.0000x reference)
"""Trainium2 Bass kernel for nn_EMAX_60756607369740.

Computation (per sample b, per group g of 16 channels, over 64x64 maps):
  - coordinate pooling strips -> 1x1 conv (w1) -> sigmoid gates -> x1
  - per-channel GroupNorm stats of x1 (used only through the a21-weighted
    channel contraction; a11 == uniform 1/16 exactly because the GN output
    has zero spatial mean)
  - 3x3 conv branch x2 enters only via (1/16)*sum_c x2 (a single
    8-output-channel conv with channel-summed weights) and via its pooled
    per-channel sums (reconstructed algebraically from row/col/corner sums)
  - wv = (1/16)sum_c x2 + sum_c b21[c] x1[c] - k ; spatial = x*sigmoid(wv)
  - channel SE on x, fuse, global SE on fused.

Sharding: pure data parallel over batch B=16 -> 2 samples per core x 8 cores.
Per-core tile: [128 partitions = 8 groups x 16 ch, 4096 = 64h x 64w].
"""

import sys

for _p in ("/opt/trn_rl_repo", "/root/.axon_site/_ro/trn_rl_repo"):
    if _p not in sys.path:
        sys.path.insert(0, _p)

import numpy as np
import ml_dtypes

import bass_rust
import concourse.bass as bass
import concourse.mybir as mybir
from concourse.tile import TileContext
from concourse.bass_utils import run_bass_kernel_spmd

F32 = mybir.dt.float32
BF16 = mybir.dt.bfloat16
AF = mybir.ActivationFunctionType
OP = mybir.AluOpType
AX = mybir.AxisListType

B, C, H, W = 16, 512, 64, 64
G, CG, R = 32, 16, 16
EPS = 1e-5
NCORES = 8
BPC = B // NCORES          # samples per core
NSLAB = C // 128           # 4 slabs of 128 channels per sample
HW = H * W                 # 4096
WP = W + 4                 # padded row length 68 (2 left, 2 right zeros)
# taps ordered dy=0 first so the first matmul in each PSUM group covers the
# full chunk (start=True clears the whole region)
TAPS = [(0, -1), (0, 0), (0, 1),
        (-1, -1), (-1, 0), (-1, 1),
        (1, -1), (1, 0), (1, 1)]

MAX_WAITS_PER_INST = 1


def _patched_drain_and_barrier(self, tick_clock, wait_clock):
    # Workaround for walrus "Too many sync wait commands" on the final tile
    # drain: split the aggregated sem waits across many drain instructions.
    drain_inst = self.nc.sync.drain()
    wait_clock.add_sem_waits(
        drain_inst.ins, bass_rust.ScopedClock({None: tick_clock.global_clock})
    )
    mi = drain_inst.ins
    si = mi.sync_info
    if si is not None and len(si.on_wait) > MAX_WAITS_PER_INST:
        waits = list(si.on_wait)
        mi.sync_info = bass_rust.SyncInfo(
            on_wait=waits[:MAX_WAITS_PER_INST], on_update=list(si.on_update)
        )
        rest = waits[MAX_WAITS_PER_INST:]
        for i in range(0, len(rest), MAX_WAITS_PER_INST):
            d2 = self.nc.sync.drain()
            d2.ins.sync_info = bass_rust.SyncInfo(
                on_wait=rest[i : i + MAX_WAITS_PER_INST], on_update=[]
            )
    self.nc.all_engine_barrier()
    popped = self.nc._tile_sem_poison_stack.pop()
    assert popped is self._sem_poison
    self.nc.clear_and_free_semaphores(list(self.sems.allocated().values()))
    self.nc.all_engine_barrier()


TileContext._drain_and_barrier = _patched_drain_and_barrier


def _split_sync_waits(nc, maxw=MAX_WAITS_PER_INST):
    """Walrus rejects instructions carrying more than a couple of sync
    waits. Rebuild each basic block, hoisting excess waits onto freshly
    created same-engine nops placed immediately before the instruction."""
    func = nc.m.functions[0]
    for blk in func.blocks:
        insts = list(blk.instructions)
        need = []
        for inst in insts:
            si = inst.sync_info
            if si is not None and len(si.on_wait) > maxw:
                need.append(inst)
        if not need:
            continue
        donors = {}
        for inst in need:
            si = inst.sync_info
            waits = list(si.on_wait)
            extra = waits[:-maxw] if maxw > 0 else waits
            keep = waits[-maxw:] if maxw > 0 else []
            inst.sync_info = bass_rust.SyncInfo(
                on_wait=keep, on_update=list(si.on_update))
            chunks = [extra[i:i + max(maxw, 1)]
                      for i in range(0, len(extra), max(maxw, 1))]
            nops = []
            for ch in chunks:
                bi = nc.engines[inst.engine].nop()
                ni = bi.ins
                ni.sync_info = bass_rust.SyncInfo(on_wait=ch, on_update=[])
                nops.append(ni)
                # the nop was appended to the current bb; pull it back out
                cur = nc.cur_bb.bb if hasattr(nc.cur_bb, "bb") else None
                for fb in func.blocks:
                    fl = list(fb.instructions)
                    if fl and fl[-1] is ni:
                        fb.instructions = fl[:-1]
                        break
            donors[id(inst)] = nops
        out = []
        for inst in insts:
            out.extend(donors.get(id(inst), []))
            out.append(inst)
        blk.instructions = out


def _bf(x):
    return np.ascontiguousarray(x.astype(ml_dtypes.bfloat16))


def _f32(x):
    return np.ascontiguousarray(x.astype(np.float32))


def build_consts(w1, b1, w3, b3, gn_w, gn_b, cg_w1, cg_b1, cg_w2, cg_b2,
                 ga_w1, ga_b1, ga_w2, ga_b2, gamma):
    """Host-side weight transforms. All arrays laid out [partition, free]."""
    c = {}
    # strip 1x1 conv, block-diagonal over 8 groups; /64 folds the W (or H) mean
    Wstrip = np.zeros((128, 128), np.float32)
    for g in range(8):
        # out[(g,o)] = sum_c w1[o,c] * strip[(g,c)] / 64
        Wstrip[g * 16:(g + 1) * 16, g * 16:(g + 1) * 16] = w1.T / 64.0
    c["Wstrip"] = _bf(Wstrip)
    c["b1t"] = _f32(np.tile(b1, 8)[:, None])

    # big conv tap weights (only used for the pooled-sum reconstruction, N=4)
    W3t = np.zeros((128, 9, 128), np.float32)
    for t, (dy, dx) in enumerate(TAPS):
        blk = w3[:, :, dy + 1, dx + 1].T  # [c_in, c_out]
        for g in range(8):
            W3t[g * 16:(g + 1) * 16, t, g * 16:(g + 1) * 16] = blk
    c["W3sb"] = _bf(W3t.reshape(128, 9 * 128))

    # channel-mean conv weights: out group column g, value sum_c w3 / 16
    w3bar = np.zeros((128, 9, 8), np.float32)
    for t, (dy, dx) in enumerate(TAPS):
        vec = w3[:, :, dy + 1, dx + 1].sum(0) / 16.0  # [c_in]
        for g in range(8):
            w3bar[g * 16:(g + 1) * 16, t, g] = vec
    c["w3bar"] = _bf(w3bar.reshape(128, 9 * 8))

    diagb3 = np.zeros((128, 128), np.float32)
    for g in range(8):
        diagb3[g * 16:(g + 1) * 16, g * 16:(g + 1) * 16] = np.diag(b3)
    c["diagb3"] = _bf(diagb3)
    c["c4096"] = _bf(np.full((128, 4), 4096.0, np.float32))

    mask = np.zeros((128, 8), np.float32)
    for g in range(8):
        mask[g * 16:(g + 1) * 16, g] = 1.0
    c["maskblk"] = _bf(mask)
    maskT = np.zeros((8, 128), np.float32)
    for g in range(8):
        maskT[g, g * 16:(g + 1) * 16] = 1.0
    c["maskT"] = _bf(maskT)
    c["magic"] = np.full((128, 1), 0x5f3759df, np.int32)

    # SE dense layers, per 128-channel slab; /4096 folds the HW mean
    cg1 = np.zeros((128, 4, 32), np.float32)
    ga1 = np.zeros((128, 4, 32), np.float32)
    cg2 = np.zeros((32, 4, 128), np.float32)
    ga2 = np.zeros((32, 4, 128), np.float32)
    for s in range(4):
        cg1[:, s, :] = cg_w1[:, s * 128:(s + 1) * 128].T / 4096.0
        ga1[:, s, :] = ga_w1[:, s * 128:(s + 1) * 128].T / 4096.0
        cg2[:, s, :] = cg_w2[s * 128:(s + 1) * 128, :].T
        ga2[:, s, :] = ga_w2[s * 128:(s + 1) * 128, :].T
    c["cg1w"] = _bf(cg1.reshape(128, 128))
    c["ga1w"] = _bf(ga1.reshape(128, 128))
    c["cg2w"] = _bf(cg2.reshape(32, 512))
    c["ga2w"] = _bf(ga2.reshape(32, 512))
    c["cgb1"] = _f32(cg_b1[:, None])
    c["gab1"] = _f32(ga_b1[:, None])
    c["cgb2"] = _f32(cg_b2.reshape(4, 128).T)
    c["gab2"] = _f32(ga_b2.reshape(4, 128).T)

    c["epsc"] = _f32(np.full((128, 1), EPS, np.float32))
    c["gnw"] = _f32(np.tile(gn_w, 8)[:, None])
    c["gnb"] = _f32(np.tile(gn_b, 8)[:, None])
    return c


def build_program(gamma_f, mean_b3_f, const_shapes):
    nc = bass.Bass("TRN2", target_bir_lowering=False, debug=False,
                   num_devices=NCORES)
    x_d = nc.dram_tensor("x", [BPC, C, H, W], F32, kind="ExternalInput")
    y_d = nc.dram_tensor("y", [BPC, C, H, W], F32, kind="ExternalOutput")
    cd = {}
    for name, (shape, dt) in const_shapes.items():
        cd[name] = nc.dram_tensor(name, list(shape), dt, kind="ExternalInput")

    with TileContext(nc) as tc:
        with (
            tc.sbuf_pool(name="consts", bufs=1) as cpool,
            tc.sbuf_pool(name="big", bufs=2) as bpool,
            tc.sbuf_pool(name="fusedp", bufs=NSLAB) as fpool,
            tc.sbuf_pool(name="small", bufs=2) as spool,
            tc.psum_pool(name="pwv", bufs=3) as pwv,
            tc.psum_pool(name="pstrip", bufs=1) as pstrip,
            tc.psum_pool(name="psmall", bufs=1) as psmall,
            tc.psum_pool(name="prep", bufs=2) as prep,
        ):
            cs = {}
            for name in const_shapes:
                t = cpool.tile(list(cd[name].shape), cd[name].dtype, name=f"c_{name}")
                nc.sync.dma_start(t[:, :], cd[name].ap())
                cs[name] = t

            for b in range(BPC):
                emit_sample(nc, tc, b, x_d, y_d, cs,
                            bpool, fpool, spool, pwv, pstrip, psmall, prep,
                            gamma_f, mean_b3_f)
    _split_sync_waits(nc)
    return nc


def emit_sample(nc, tc, b, x_d, y_d, cs, bpool, fpool, spool,
                pwv, pstrip, psmall, prep, gamma_f, mean_b3_f):
    sc = nc.scalar
    ve = nc.vector
    gp = nc.gpsimd
    te = nc.tensor

    # per-sample small stats tiles ([*, slab] columns)
    xsum = spool.tile([128, 4], F32, name="xsum")
    x1sum = spool.tile([128, 4], F32, name="x1sum")
    x1sq = spool.tile([128, 4], F32, name="x1sq")
    fsum = spool.tile([128, 4], F32, name="fsum")
    edges = spool.tile([128, 16], F32, name="edges")
    corners = spool.tile([128, 16], F32, name="corners")
    mu = spool.tile([128, 4], F32, name="mu")
    rstdw = spool.tile([128, 4], F32, name="rstdw")
    b21 = spool.tile([128, 4], F32, name="b21")
    biaswv = spool.tile([8, 4], F32, name="biaswv")
    scb = spool.tile([128, 4], F32, name="scb")
    gga = spool.tile([128, 4], F32, name="gga")

    def custom_ap(base_ap, extra_off, free_dims):
        p0 = list(base_ap.ap[0])
        return bass.AP(base_ap.tensor, base_ap.offset + extra_off,
                       [p0] + [list(d) for d in free_dims])

    xpads = []
    stripss = []
    # ---------------- per-slab prefix: load, pad-convert, pooled sums ------
    for s in range(NSLAB):
        xpad = bpool.tile([128, H * WP], BF16, name="xpad", tag="xpad", bufs=4)
        strips = spool.tile([128, 128], F32, name="strips", tag="strips",
                            bufs=5)
        xpads.append(xpad)
        stripss.append(strips)

        src = x_d.ap()[b, s * 128:(s + 1) * 128, :, :]
        src = bass.AP(src.tensor, src.offset,
                      [list(src.ap[0]), [64, 64], [1, 64]])

        xp3 = xpad[:, :].rearrange("p (h w) -> p h w", w=WP)
        gp.memset(xp3[:, :, 0:2], 0.0)
        gp.memset(xp3[:, :, W + 2:W + 4], 0.0)
        gp.dma_start(xp3[:, :, 2:W + 2], src)
        xpi3 = xp3[:, :, 2:W + 2]

        ve.tensor_reduce(strips[:, 0:64], xpi3, AX.X, OP.add)
        ve.tensor_reduce(strips[:, 64:128], xpi3.transpose([0, 2, 1]), AX.X,
                         OP.add)
        ve.tensor_reduce(xsum[:, s:s + 1], strips[:, 0:64], AX.X, OP.add)
        eap = custom_ap(strips[:, :], 0, [[64, 2], [63, 2]])
        gp.tensor_copy(
            edges[:, 4 * s:4 * s + 4].rearrange("p (a c) -> p a c", c=2), eap)
        cap = custom_ap(xpad[:, :], 2, [[63 * WP, 2], [63, 2]])
        gp.tensor_copy(
            corners[:, 4 * s:4 * s + 4].rearrange("p (a c) -> p a c", c=2),
            cap)

    # ---------------- channel SE (needs all 4 slab xsums) ------------------
    xsumbf = spool.tile([128, 4], BF16, name="xsumbf")
    sc.activation(xsumbf[:, :], xsum[:, :], AF.Copy)
    h1 = psmall.tile([32, 1], F32, name="h1", tag="ps_a")
    for s in range(NSLAB):
        te.matmul(h1[:, :], cs["cg1w"][:, 32 * s:32 * s + 32],
                  xsumbf[:, s:s + 1], start=(s == 0), stop=(s == 3))
    hid = spool.tile([32, 1], BF16, name="hid", tag="hid")
    sc.activation(hid[:, :], h1[:, :], AF.Relu, bias=cs["cgb1"][:, 0:1])
    gps = psmall.tile([128, 4], F32, name="gps", tag="ps_b")
    for s in range(NSLAB):
        te.matmul(gps[:, s:s + 1], cs["cg2w"][:, 128 * s:128 * s + 128],
                  hid[:, :], start=True, stop=True)
    for s in range(NSLAB):
        sc.activation(scb[:, s:s + 1], gps[:, s:s + 1], AF.Sigmoid,
                      bias=cs["cgb2"][:, s:s + 1])
    ve.tensor_scalar_mul(scb[:, :], scb[:, :], float(gamma_f))

    # ---------------- per-slab main chain ---------------------------------
    fuseds = []
    for s in range(NSLAB):
        xpad = xpads[s]
        strips = stripss[s]
        xp3 = xpad[:, :].rearrange("p (h w) -> p h w", w=WP)
        xpi3 = xp3[:, :, 2:W + 2]

        stripbf = spool.tile([128, 128], BF16, name="stripbf", tag="stripbf")
        sc.activation(stripbf[:, :], strips[:, :], AF.Copy)
        strip_ps = pstrip.tile([128, 128], F32, name="strip_ps", tag="strip")
        te.matmul(strip_ps[:, :], cs["Wstrip"][:, :], stripbf[:, :],
                  start=True, stop=True)
        sgate = spool.tile([128, 128], BF16, name="sgate", tag="sgate",
                           bufs=3)
        sc.activation(sgate[:, :], strip_ps[:, :], AF.Sigmoid,
                      bias=cs["b1t"][:, 0:1])

        # x1 = x * sig(xh) * sig(xw); w-gate first (bcast on outer dim)
        x1 = bpool.tile([128, HW], BF16, name="x1", tag="x1", bufs=3)
        x13 = x1[:, :].rearrange("p (h w) -> p h w", w=W)
        wg = sgate[:, 64:128].unsqueeze(1).broadcast_to((128, H, W))
        gp.tensor_tensor(x13, xpi3, wg, OP.mult)
        hg = sgate[:, 0:64].unsqueeze(2).broadcast_to((128, H, W))
        ve.scalar_tensor_tensor(x13, x13, 0.0, hg, OP.add, OP.mult,
                                accum_out=x1sum[:, s:s + 1])
        junk = bpool.tile([128, HW], BF16, name="junk", tag="scratch",
                          bufs=3)
        sc.activation(junk[:, :], x1[:, :], AF.Square,
                      accum_out=x1sq[:, s:s + 1])

        # GN stats -> rstd (= exp(-0.5 ln(var+eps))) * gn_w
        gp.tensor_scalar_mul(mu[:, s:s + 1], x1sum[:, s:s + 1], 1.0 / HW)
        var = spool.tile([128, 1], F32, name="var", tag="var")
        gp.tensor_tensor(var[:, :], mu[:, s:s + 1], mu[:, s:s + 1], OP.mult)
        ve.scalar_tensor_tensor(var[:, :], x1sq[:, s:s + 1], 1.0 / HW,
                                var[:, :], OP.mult, OP.subtract)
        ve.tensor_scalar_add(var[:, :], var[:, :], float(EPS))
        # rstd = rsqrt(var) via bit trick + 2 Newton iterations
        ti = spool.tile([128, 1], mybir.dt.int32, name="ti", tag="ti")
        ve.tensor_scalar(ti[:, :], var[:, :].bitcast(mybir.dt.int32), 1,
                         None, OP.logical_shift_right)
        ve.tensor_tensor(ti[:, :], cs["magic"][:, 0:1], ti[:, :], OP.subtract)
        ry = spool.tile([128, 1], F32, name="ry", tag="ry")
        rt = spool.tile([128, 1], F32, name="rt", tag="rt")
        ve.tensor_copy(ry[:, :], ti[:, :].bitcast(F32))
        for _ in range(2):
            gp.tensor_tensor(rt[:, :], ry[:, :], ry[:, :], OP.mult)
            gp.tensor_tensor(rt[:, :], rt[:, :], var[:, :], OP.mult)
            gp.tensor_scalar(rt[:, :], rt[:, :], -0.5, 1.5, OP.mult, OP.add)
            gp.tensor_tensor(ry[:, :], ry[:, :], rt[:, :], OP.mult)
        gp.tensor_tensor(rstdw[:, s:s + 1], ry[:, :], cs["gnw"][:, 0:1],
                         OP.mult)

        # pooled x2 sums via edge algebra: A[p, tap]
        A = spool.tile([128, 9], F32, name="A", tag="A")
        Ap = A[:, :]
        gp.tensor_copy(Ap, xsum[:, s:s + 1].broadcast_to((128, 9)))
        out_r = custom_ap(Ap, 3, [[3, 2], [1, 3]])
        in_r = custom_ap(edges[:, :], 4 * s + 1, [[-1, 2], [0, 3]])
        gp.tensor_tensor(out_r, out_r, in_r, OP.subtract)
        out_c1 = custom_ap(Ap, 0, [[3, 3]])
        in_c1 = custom_ap(edges[:, :], 4 * s + 3, [[0, 3]])
        gp.tensor_tensor(out_c1, out_c1, in_c1, OP.subtract)
        out_c2 = custom_ap(Ap, 2, [[3, 3]])
        in_c2 = custom_ap(edges[:, :], 4 * s + 2, [[0, 3]])
        gp.tensor_tensor(out_c2, out_c2, in_c2, OP.subtract)
        out_k1 = custom_ap(Ap, 3, [[2, 2]])
        in_k1 = custom_ap(corners[:, :], 4 * s + 3, [[-1, 2]])
        gp.tensor_tensor(out_k1, out_k1, in_k1, OP.add)
        out_k2 = custom_ap(Ap, 6, [[2, 2]])
        in_k2 = custom_ap(corners[:, :], 4 * s + 1, [[-1, 2]])
        gp.tensor_tensor(out_k2, out_k2, in_k2, OP.add)
        Abf = spool.tile([128, 9], BF16, name="Abf", tag="Abf")
        sc.activation(Abf[:, :], A[:, :], AF.Copy)

        p2 = psmall.tile([128, 1], F32, name="p2", tag="ps_a")
        for t in range(9):
            te.matmul(p2[:, :], cs["W3sb"][:, 128 * t:128 * t + 128],
                      Abf[:, t:t + 1], start=(t == 0), stop=False)
        te.matmul(p2[:, :], cs["diagb3"][:, :], cs["c4096"][:, 0:1],
                  start=False, stop=True)
        m = spool.tile([128, 1], F32, name="m", tag="m")
        sc.activation(m[:, :], p2[:, :], AF.Copy, scale=1.0 / HW)
        # exp(m) by degree-4 Taylor (|m| << 1): e = 1+m(1+m(1/2+m(1/6+m/24)))
        e4 = spool.tile([128, 1], F32, name="e4", tag="e4")
        gp.tensor_scalar(e4[:, :], m[:, :], 1.0 / 24, 1.0 / 6, OP.mult,
                         OP.add)
        gp.tensor_tensor(e4[:, :], e4[:, :], m[:, :], OP.mult)
        gp.tensor_scalar_add(e4[:, :], e4[:, :], 0.5)
        gp.tensor_tensor(e4[:, :], e4[:, :], m[:, :], OP.mult)
        gp.tensor_scalar_add(e4[:, :], e4[:, :], 1.0)
        gp.tensor_tensor(e4[:, :], e4[:, :], m[:, :], OP.mult)
        gp.tensor_scalar_add(e4[:, :], e4[:, :], 1.0)
        ebf = spool.tile([128, 1], BF16, name="ebf", tag="ebf")
        sc.activation(ebf[:, :], e4[:, :], AF.Copy)
        s8 = psmall.tile([8, 1], F32, name="s8", tag="ps_b")
        te.matmul(s8[:, :], cs["maskblk"][:, :], ebf[:, :], start=True,
                  stop=True)
        s8bf = spool.tile([8, 1], BF16, name="s8bf", tag="s8bf")
        sc.activation(s8bf[:, :], s8[:, :], AF.Copy)
        rs8 = psmall.tile([128, 1], F32, name="rs8", tag="ps_b")
        te.matmul(rs8[:, :], cs["maskT"][:, :], s8bf[:, :], start=True,
                  stop=True)
        rec = spool.tile([128, 1], F32, name="rec", tag="rec")
        ve.reciprocal(rec[:, :], rs8[:, :])
        a21 = spool.tile([128, 1], F32, name="a21", tag="a21")
        gp.tensor_tensor(a21[:, :], e4[:, :], rec[:, :], OP.mult)
        gp.tensor_tensor(b21[:, s:s + 1], a21[:, :], rstdw[:, s:s + 1],
                         OP.mult)
        kv = spool.tile([128, 1], F32, name="kv", tag="kv")
        gp.tensor_tensor(kv[:, :], b21[:, s:s + 1], mu[:, s:s + 1], OP.mult)
        k2bf = spool.tile([128, 1], BF16, name="k2bf", tag="k2bf")
        ve.scalar_tensor_tensor(k2bf[:, :], a21[:, :], cs["gnb"][:, 0:1],
                                kv[:, :], OP.mult, OP.subtract)
        kps = psmall.tile([8, 1], F32, name="kps", tag="ps_a")
        te.matmul(kps[:, :], cs["maskblk"][:, :], k2bf[:, :], start=True,
                  stop=True)
        sc.activation(biaswv[:, s:s + 1], kps[:, :], AF.Copy,
                      bias=float(mean_b3_f))
        b21blk = spool.tile([128, 8], BF16, name="b21blk", tag="b21blk",
                            bufs=3)
        ve.tensor_tensor(b21blk[:, :], cs["maskblk"][:, :],
                         b21[:, s:s + 1].broadcast_to((128, 8)), OP.mult)

        # wv = mean_c(conv) + sum_c b21 x1 ; sigmoid with per-group bias
        sig = bpool.tile([8, HW], BF16, name="sig", tag="scratch", bufs=3)
        for half in range(2):
            wv = pwv.tile([128, 512], F32, name="wv", tag="wv")
            for t, (dy, dx) in enumerate(TAPS):
                for q in range(4):
                    hc = 4 * half + q
                    h0 = hc * 8
                    i0 = max(0, -(h0 + dy))
                    i1 = min(8, 64 - h0 - dy)
                    rhs = xp3[:, h0 + i0 + dy:h0 + i1 + dy,
                              2 + dx:2 + dx + W]
                    te.matmul(wv[32 * q:32 * q + 8, i0 * 64:i1 * 64],
                              cs["w3bar"][:, 8 * t:8 * t + 8], rhs,
                              start=(t == 0), stop=False,
                              tile_position=(0, 32 * q))
            for q in range(4):
                hc = 4 * half + q
                te.matmul(wv[32 * q:32 * q + 8, :], b21blk[:, :],
                          x1[:, 512 * hc:512 * hc + 512], start=False,
                          stop=True, tile_position=(0, 32 * q))
            for q in range(4):
                hc = 4 * half + q
                sc.activation(sig[:, 512 * hc:512 * hc + 512],
                              wv[32 * q:32 * q + 8, :],
                              AF.Sigmoid, bias=biaswv[:, s:s + 1])
        fused = fpool.tile([128, HW], F32, name="fused", tag="fused")
        fuseds.append(fused)
        fparts = spool.tile([128, 8], F32, name="fparts", tag="fparts")
        for hc in range(8):
            repq = prep.tile([128, 512], F32, name="repq", tag="repq")
            te.matmul(repq[:, :], cs["maskT"][:, :],
                      sig[:, 512 * hc:512 * hc + 512], start=True, stop=True)
            ve.scalar_tensor_tensor(
                fused[:, 512 * hc:512 * hc + 512].rearrange(
                    "p (h w) -> p h w", w=W),
                repq[:, :].rearrange("p (h w) -> p h w", w=W),
                scb[:, s:s + 1], xpi3[:, 8 * hc:8 * hc + 8, :],
                OP.add, OP.mult, accum_out=fparts[:, hc:hc + 1])
        ve.tensor_reduce(fsum[:, s:s + 1], fparts[:, :], AX.X, OP.add)

    # ---------------- global-attn SE over fused ---------------------------
    fsumbf = spool.tile([128, 4], BF16, name="fsumbf")
    sc.activation(fsumbf[:, :], fsum[:, :], AF.Copy)
    h2 = psmall.tile([32, 1], F32, name="h2", tag="ps_a")
    for s in range(NSLAB):
        te.matmul(h2[:, :], cs["ga1w"][:, 32 * s:32 * s + 32],
                  fsumbf[:, s:s + 1], start=(s == 0), stop=(s == 3))
    hid2 = spool.tile([32, 1], BF16, name="hid2", tag="hid")
    sc.activation(hid2[:, :], h2[:, :], AF.Relu, bias=cs["gab1"][:, 0:1])
    gps2 = psmall.tile([128, 4], F32, name="gps2", tag="ps_b")
    for s in range(NSLAB):
        te.matmul(gps2[:, s:s + 1], cs["ga2w"][:, 128 * s:128 * s + 128],
                  hid2[:, :], start=True, stop=True)
    for s in range(NSLAB):
        sc.activation(gga[:, s:s + 1], gps2[:, s:s + 1], AF.Sigmoid,
                      bias=cs["gab2"][:, s:s + 1])

    # ---------------- final gate + store -----------------------------------
    for s in range(NSLAB):
        fused = fuseds[s]
        sc.activation(fused[:, :], fused[:, :], AF.Copy,
                      scale=gga[:, s:s + 1])
        dst = y_d.ap()[b, s * 128:(s + 1) * 128, :, :]
        dst = bass.AP(dst.tensor, dst.offset, [list(dst.ap[0]), [1, HW]])
        nc.sync.dma_start(dst, fused[:, :])


def _ensure_ntff_hook():
    """run_bass_kernel_spmd(trace=True) under axon needs
    antenv.axon_hooks, which this image's antenv lacks. Shim it and
    register the ctypes-based NTFF hook from the boot package."""
    import types
    try:
        from antenv import axon_hooks  # noqa: F401
        return
    except ImportError:
        pass
    try:
        import antenv
        from trn_agent_boot.trn_boot import _ntff_profile_via_ctypes
        hooks = types.ModuleType("antenv.axon_hooks")
        _h = [None]
        hooks.set_axon_ntff_profile_hook = lambda h: _h.__setitem__(0, h)
        hooks.get_axon_ntff_profile_hook = lambda: _h[0]
        sys.modules["antenv.axon_hooks"] = hooks
        antenv.axon_hooks = hooks
        hooks.set_axon_ntff_profile_hook(
            _ntff_profile_via_ctypes("/opt/axon/libaxon_pjrt.so"))
    except Exception as e:  # profiling is best-effort
        print(f"ntff hook setup failed: {e}")


_CACHE = {}


def _get_program(consts, gamma_f, mean_b3_f):
    key = (float(gamma_f), float(mean_b3_f),
           tuple(sorted((k, v.tobytes()[:64].hex() if v.size > 16 else
                         v.tobytes().hex()) for k, v in consts.items())))
    key = hash(key)
    if key not in _CACHE:
        def _dt(v):
            if v.dtype == ml_dtypes.bfloat16:
                return BF16
            if v.dtype == np.int32:
                return mybir.dt.int32
            return F32
        const_shapes = {k: (v.shape, _dt(v)) for k, v in consts.items()}
        _CACHE[key] = build_program(gamma_f, mean_b3_f, const_shapes)
    return _CACHE[key]


def kernel(x, w1, b1, w3, b3, gn_w, gn_b, cg_w1, cg_b1, cg_w2, cg_b2,
           ga_w1, ga_b1, ga_w2, ga_b2, gamma, _return_timing=None):
    args = [np.asarray(a) for a in
            (x, w1, b1, w3, b3, gn_w, gn_b, cg_w1, cg_b1, cg_w2, cg_b2,
             ga_w1, ga_b1, ga_w2, ga_b2, gamma)]
    (x, w1, b1, w3, b3, gn_w, gn_b, cg_w1, cg_b1, cg_w2, cg_b2,
     ga_w1, ga_b1, ga_w2, ga_b2, gamma) = args
    consts = build_consts(w1, b1, w3, b3, gn_w, gn_b, cg_w1, cg_b1, cg_w2,
                          cg_b2, ga_w1, ga_b1, ga_w2, ga_b2, gamma)
    gamma_f = float(np.asarray(gamma).reshape(-1)[0])
    mean_b3_f = float(np.mean(b3))
    nc = _get_program(consts, gamma_f, mean_b3_f)

    in_maps = []
    for core in range(NCORES):
        m = {"x": _f32(x[core * BPC:(core + 1) * BPC])}
        m.update(consts)
        in_maps.append(m)
    trace = bool(_return_timing is not None)
    if trace:
        _ensure_ntff_hook()
    res = run_bass_kernel_spmd(nc, in_maps, core_ids=list(range(NCORES)),
                               trace=trace)
    if _return_timing is not None:
        _return_timing.update(dict(
            exec_time_ns=res.exec_time_ns,
            mean_exec_time_ns=res.mean_exec_time_ns,
        ))
    out = np.empty((B, C, H, W), np.float32)
    for core in range(NCORES):
        out[core * BPC:(core + 1) * BPC] = res.results[core]["y"]
    return out


# revision 12
# speedup vs baseline: 1.0566x; 1.0566x over previous
"""Trainium2 Bass kernel for nn_EMAX_60756607369740.

Computation (per sample b, per group g of 16 channels, over 64x64 maps):
  - coordinate pooling strips -> 1x1 conv (w1) -> sigmoid gates -> x1
  - per-channel GroupNorm stats of x1 (used only through the a21-weighted
    channel contraction; a11 == uniform 1/16 exactly because the GN output
    has zero spatial mean)
  - 3x3 conv branch x2 enters only via (1/16)*sum_c x2 (a single
    8-output-channel conv with channel-summed weights) and via its pooled
    per-channel sums (reconstructed algebraically from row/col/corner sums)
  - wv = (1/16)sum_c x2 + sum_c b21[c] x1[c] - k ; spatial = x*sigmoid(wv)
  - channel SE on x, fuse, global SE on fused.

Sharding: pure data parallel over batch B=16 -> 2 samples per core x 8 cores.
Per-core tile: [128 partitions = 8 groups x 16 ch, 4096 = 64h x 64w].
"""

import sys

for _p in ("/opt/trn_rl_repo", "/root/.axon_site/_ro/trn_rl_repo"):
    if _p not in sys.path:
        sys.path.insert(0, _p)

import numpy as np
import ml_dtypes

import bass_rust
import concourse.bass as bass
import concourse.mybir as mybir
from concourse.tile import TileContext
from concourse.bass_utils import run_bass_kernel_spmd

F32 = mybir.dt.float32
BF16 = mybir.dt.bfloat16
AF = mybir.ActivationFunctionType
OP = mybir.AluOpType
AX = mybir.AxisListType

B, C, H, W = 16, 512, 64, 64
G, CG, R = 32, 16, 16
EPS = 1e-5
NCORES = 8
BPC = B // NCORES          # samples per core
NSLAB = C // 128           # 4 slabs of 128 channels per sample
HW = H * W                 # 4096
WP = W + 4                 # padded row length 68 (2 left, 2 right zeros)
# taps ordered dy=0 first so the first matmul in each PSUM group covers the
# full chunk (start=True clears the whole region)
TAPS = [(0, -1), (0, 0), (0, 1),
        (-1, -1), (-1, 0), (-1, 1),
        (1, -1), (1, 0), (1, 1)]

MAX_WAITS_PER_INST = 1


def _patched_drain_and_barrier(self, tick_clock, wait_clock):
    # Workaround for walrus "Too many sync wait commands" on the final tile
    # drain: split the aggregated sem waits across many drain instructions.
    drain_inst = self.nc.sync.drain()
    wait_clock.add_sem_waits(
        drain_inst.ins, bass_rust.ScopedClock({None: tick_clock.global_clock})
    )
    mi = drain_inst.ins
    si = mi.sync_info
    if si is not None and len(si.on_wait) > MAX_WAITS_PER_INST:
        waits = list(si.on_wait)
        mi.sync_info = bass_rust.SyncInfo(
            on_wait=waits[:MAX_WAITS_PER_INST], on_update=list(si.on_update)
        )
        rest = waits[MAX_WAITS_PER_INST:]
        for i in range(0, len(rest), MAX_WAITS_PER_INST):
            d2 = self.nc.sync.drain()
            d2.ins.sync_info = bass_rust.SyncInfo(
                on_wait=rest[i : i + MAX_WAITS_PER_INST], on_update=[]
            )
    self.nc.all_engine_barrier()
    popped = self.nc._tile_sem_poison_stack.pop()
    assert popped is self._sem_poison
    self.nc.clear_and_free_semaphores(list(self.sems.allocated().values()))
    self.nc.all_engine_barrier()


TileContext._drain_and_barrier = _patched_drain_and_barrier


def _split_sync_waits(nc, maxw=MAX_WAITS_PER_INST):
    """Walrus rejects instructions carrying more than a couple of sync
    waits. Rebuild each basic block, hoisting excess waits onto freshly
    created same-engine nops placed immediately before the instruction."""
    func = nc.m.functions[0]
    for blk in func.blocks:
        insts = list(blk.instructions)
        need = []
        for inst in insts:
            si = inst.sync_info
            if si is not None and len(si.on_wait) > maxw:
                need.append(inst)
        if not need:
            continue
        donors = {}
        for inst in need:
            si = inst.sync_info
            waits = list(si.on_wait)
            extra = waits[:-maxw] if maxw > 0 else waits
            keep = waits[-maxw:] if maxw > 0 else []
            inst.sync_info = bass_rust.SyncInfo(
                on_wait=keep, on_update=list(si.on_update))
            chunks = [extra[i:i + max(maxw, 1)]
                      for i in range(0, len(extra), max(maxw, 1))]
            nops = []
            for ch in chunks:
                bi = nc.engines[inst.engine].nop()
                ni = bi.ins
                ni.sync_info = bass_rust.SyncInfo(on_wait=ch, on_update=[])
                nops.append(ni)
                # the nop was appended to the current bb; pull it back out
                cur = nc.cur_bb.bb if hasattr(nc.cur_bb, "bb") else None
                for fb in func.blocks:
                    fl = list(fb.instructions)
                    if fl and fl[-1] is ni:
                        fb.instructions = fl[:-1]
                        break
            donors[id(inst)] = nops
        out = []
        for inst in insts:
            out.extend(donors.get(id(inst), []))
            out.append(inst)
        blk.instructions = out


def _bf(x):
    return np.ascontiguousarray(x.astype(ml_dtypes.bfloat16))


def _f32(x):
    return np.ascontiguousarray(x.astype(np.float32))


def build_consts(w1, b1, w3, b3, gn_w, gn_b, cg_w1, cg_b1, cg_w2, cg_b2,
                 ga_w1, ga_b1, ga_w2, ga_b2, gamma):
    """Host-side weight transforms. All arrays laid out [partition, free]."""
    c = {}
    # strip 1x1 conv, block-diagonal over 8 groups; /64 folds the W (or H) mean
    Wstrip = np.zeros((128, 128), np.float32)
    for g in range(8):
        # out[(g,o)] = sum_c w1[o,c] * strip[(g,c)] / 64
        Wstrip[g * 16:(g + 1) * 16, g * 16:(g + 1) * 16] = w1.T / 64.0
    c["Wstrip"] = _bf(Wstrip)
    c["b1t"] = _f32(np.tile(b1, 8)[:, None])

    # big conv tap weights (only used for the pooled-sum reconstruction, N=4)
    W3t = np.zeros((128, 9, 128), np.float32)
    for t, (dy, dx) in enumerate(TAPS):
        blk = w3[:, :, dy + 1, dx + 1].T  # [c_in, c_out]
        for g in range(8):
            W3t[g * 16:(g + 1) * 16, t, g * 16:(g + 1) * 16] = blk
    c["W3sb"] = _bf(W3t.reshape(128, 9 * 128))

    # channel-mean conv weights: out group column g, value sum_c w3 / 16
    w3bar = np.zeros((128, 9, 8), np.float32)
    for t, (dy, dx) in enumerate(TAPS):
        vec = w3[:, :, dy + 1, dx + 1].sum(0) / 16.0  # [c_in]
        for g in range(8):
            w3bar[g * 16:(g + 1) * 16, t, g] = vec
    c["w3bar"] = _bf(w3bar.reshape(128, 9 * 8))

    diagb3 = np.zeros((128, 128), np.float32)
    for g in range(8):
        diagb3[g * 16:(g + 1) * 16, g * 16:(g + 1) * 16] = np.diag(b3)
    c["diagb3"] = _bf(diagb3)
    c["c4096"] = _bf(np.full((128, 4), 4096.0, np.float32))

    mask = np.zeros((128, 8), np.float32)
    for g in range(8):
        mask[g * 16:(g + 1) * 16, g] = 1.0
    c["maskblk"] = _bf(mask)
    maskT = np.zeros((8, 128), np.float32)
    for g in range(8):
        maskT[g, g * 16:(g + 1) * 16] = 1.0
    c["maskT"] = _bf(maskT)
    c["magic"] = np.full((128, 1), 0x5f3759df, np.int32)

    # SE dense layers, per 128-channel slab; /4096 folds the HW mean
    cg1 = np.zeros((128, 4, 32), np.float32)
    ga1 = np.zeros((128, 4, 32), np.float32)
    cg2 = np.zeros((32, 4, 128), np.float32)
    ga2 = np.zeros((32, 4, 128), np.float32)
    for s in range(4):
        cg1[:, s, :] = cg_w1[:, s * 128:(s + 1) * 128].T / 4096.0
        ga1[:, s, :] = ga_w1[:, s * 128:(s + 1) * 128].T / 4096.0
        cg2[:, s, :] = cg_w2[s * 128:(s + 1) * 128, :].T
        ga2[:, s, :] = ga_w2[s * 128:(s + 1) * 128, :].T
    c["cg1w"] = _bf(cg1.reshape(128, 128))
    c["ga1w"] = _bf(ga1.reshape(128, 128))
    c["cg2w"] = _bf(cg2.reshape(32, 512))
    c["ga2w"] = _bf(ga2.reshape(32, 512))
    c["cgb1"] = _f32(cg_b1[:, None])
    c["gab1"] = _f32(ga_b1[:, None])
    c["cgb2"] = _f32(cg_b2.reshape(4, 128).T)
    c["gab2"] = _f32(ga_b2.reshape(4, 128).T)

    c["epsc"] = _f32(np.full((128, 1), EPS, np.float32))
    c["gnw"] = _f32(np.tile(gn_w, 8)[:, None])
    c["gnb"] = _f32(np.tile(gn_b, 8)[:, None])
    return c


def build_program(gamma_f, mean_b3_f, const_shapes):
    nc = bass.Bass("TRN2", target_bir_lowering=False, debug=False,
                   num_devices=NCORES)
    x_d = nc.dram_tensor("x", [BPC, C, H, W], F32, kind="ExternalInput")
    y_d = nc.dram_tensor("y", [BPC, C, H, W], F32, kind="ExternalOutput")
    cd = {}
    for name, (shape, dt) in const_shapes.items():
        cd[name] = nc.dram_tensor(name, list(shape), dt, kind="ExternalInput")

    with TileContext(nc) as tc:
        with (
            tc.sbuf_pool(name="consts", bufs=1) as cpool,
            tc.sbuf_pool(name="big", bufs=2) as bpool,
            tc.sbuf_pool(name="fusedp", bufs=NSLAB + 1) as fpool,
            tc.sbuf_pool(name="small", bufs=2) as spool,
            tc.psum_pool(name="pwv", bufs=2) as pwv,
            tc.psum_pool(name="pstrip", bufs=1) as pstrip,
            tc.psum_pool(name="psmall", bufs=2) as psmall,
            tc.psum_pool(name="prep", bufs=2) as prep,
        ):
            cs = {}
            for name in const_shapes:
                t = cpool.tile(list(cd[name].shape), cd[name].dtype, name=f"c_{name}")
                nc.sync.dma_start(t[:, :], cd[name].ap())
                cs[name] = t

            for b in range(BPC):
                emit_sample(nc, tc, b, x_d, y_d, cs,
                            bpool, fpool, spool, pwv, pstrip, psmall, prep,
                            gamma_f, mean_b3_f)
    _split_sync_waits(nc)
    return nc


def emit_sample(nc, tc, b, x_d, y_d, cs, bpool, fpool, spool,
                pwv, pstrip, psmall, prep, gamma_f, mean_b3_f):
    sc = nc.scalar
    ve = nc.vector
    gp = nc.gpsimd
    te = nc.tensor

    # per-sample small stats tiles ([*, slab] columns)
    xsum = spool.tile([128, 4], F32, name="xsum")
    x1sum = spool.tile([128, 4], F32, name="x1sum")
    x1sq = spool.tile([128, 4], F32, name="x1sq")
    fsum = spool.tile([128, 4], F32, name="fsum")
    edges = spool.tile([128, 16], F32, name="edges")
    corners = spool.tile([128, 16], F32, name="corners")
    mu = spool.tile([128, 4], F32, name="mu")
    rstdw = spool.tile([128, 4], F32, name="rstdw")
    b21 = spool.tile([128, 4], F32, name="b21")
    biaswv = spool.tile([8, 4], F32, name="biaswv")
    scb = spool.tile([128, 4], F32, name="scb")
    gga = spool.tile([128, 4], F32, name="gga")

    def custom_ap(base_ap, extra_off, free_dims):
        p0 = list(base_ap.ap[0])
        return bass.AP(base_ap.tensor, base_ap.offset + extra_off,
                       [p0] + [list(d) for d in free_dims])

    xpads = []
    stripss = []
    # ---------------- per-slab prefix: load, pad-convert, pooled sums ------
    for s in range(NSLAB):
        xpad = bpool.tile([128, H * WP], BF16, name="xpad", tag="xpad", bufs=5)
        strips = spool.tile([128, 128], F32, name="strips", tag="strips",
                            bufs=5)
        xpads.append(xpad)
        stripss.append(strips)

        src = x_d.ap()[b, s * 128:(s + 1) * 128, :, :]
        src = bass.AP(src.tensor, src.offset,
                      [list(src.ap[0]), [64, 64], [1, 64]])

        xp3 = xpad[:, :].rearrange("p (h w) -> p h w", w=WP)
        gp.memset(xp3[:, :, 0:2], 0.0)
        gp.memset(xp3[:, :, W + 2:W + 4], 0.0)
        gp.dma_start(xp3[:, :, 2:W + 2], src)
        xpi3 = xp3[:, :, 2:W + 2]

        ve.tensor_reduce(strips[:, 0:64], xpi3, AX.X, OP.add)
        ve.tensor_reduce(strips[:, 64:128], xpi3.transpose([0, 2, 1]), AX.X,
                         OP.add)
        ve.tensor_reduce(xsum[:, s:s + 1], strips[:, 0:64], AX.X, OP.add)
        eap = custom_ap(strips[:, :], 0, [[64, 2], [63, 2]])
        gp.tensor_copy(
            edges[:, 4 * s:4 * s + 4].rearrange("p (a c) -> p a c", c=2), eap)
        cap = custom_ap(xpad[:, :], 2, [[63 * WP, 2], [63, 2]])
        gp.tensor_copy(
            corners[:, 4 * s:4 * s + 4].rearrange("p (a c) -> p a c", c=2),
            cap)

    # ---------------- channel SE (needs all 4 slab xsums) ------------------
    xsumbf = spool.tile([128, 4], BF16, name="xsumbf")
    sc.activation(xsumbf[:, :], xsum[:, :], AF.Copy)
    h1 = psmall.tile([32, 1], F32, name="h1", tag="ps_a")
    for s in range(NSLAB):
        te.matmul(h1[:, :], cs["cg1w"][:, 32 * s:32 * s + 32],
                  xsumbf[:, s:s + 1], start=(s == 0), stop=(s == 3))
    hid = spool.tile([32, 1], BF16, name="hid", tag="hid")
    sc.activation(hid[:, :], h1[:, :], AF.Relu, bias=cs["cgb1"][:, 0:1])
    gps = psmall.tile([128, 4], F32, name="gps", tag="ps_b", bufs=1)
    for s in range(NSLAB):
        te.matmul(gps[:, s:s + 1], cs["cg2w"][:, 128 * s:128 * s + 128],
                  hid[:, :], start=True, stop=True)
    for s in range(NSLAB):
        sc.activation(scb[:, s:s + 1], gps[:, s:s + 1], AF.Sigmoid,
                      bias=cs["cgb2"][:, s:s + 1])
    ve.tensor_scalar_mul(scb[:, :], scb[:, :], float(gamma_f))

    # ---------------- per-slab main chain ---------------------------------
    fuseds = []
    for s in range(NSLAB):
        xpad = xpads[s]
        strips = stripss[s]
        xp3 = xpad[:, :].rearrange("p (h w) -> p h w", w=WP)
        xpi3 = xp3[:, :, 2:W + 2]

        stripbf = spool.tile([128, 128], BF16, name="stripbf", tag="stripbf")
        sc.activation(stripbf[:, :], strips[:, :], AF.Copy)
        strip_ps = pstrip.tile([128, 128], F32, name="strip_ps", tag="strip")
        te.matmul(strip_ps[:, :], cs["Wstrip"][:, :], stripbf[:, :],
                  start=True, stop=True)
        sgate = spool.tile([128, 128], BF16, name="sgate", tag="sgate",
                           bufs=3)
        sc.activation(sgate[:, :], strip_ps[:, :], AF.Sigmoid,
                      bias=cs["b1t"][:, 0:1])

        # x1 = x * sig(xh) * sig(xw); w-gate first (bcast on outer dim)
        x1 = bpool.tile([128, HW], BF16, name="x1", tag="x1", bufs=3)
        x13 = x1[:, :].rearrange("p (h w) -> p h w", w=W)
        wg = sgate[:, 64:128].unsqueeze(1).broadcast_to((128, H, W))
        ve.tensor_tensor(x13, xpi3, wg, OP.mult)
        hg = sgate[:, 0:64].unsqueeze(2).broadcast_to((128, H, W))
        ve.scalar_tensor_tensor(x13, x13, 0.0, hg, OP.add, OP.mult,
                                accum_out=x1sum[:, s:s + 1])
        junk = bpool.tile([128, HW], BF16, name="junk", tag="scratch",
                          bufs=3)
        sc.activation(junk[:, :], x1[:, :], AF.Square,
                      accum_out=x1sq[:, s:s + 1])

        # GN stats -> rstd (= exp(-0.5 ln(var+eps))) * gn_w
        gp.tensor_scalar_mul(mu[:, s:s + 1], x1sum[:, s:s + 1], 1.0 / HW)
        var = spool.tile([128, 1], F32, name="var", tag="var")
        gp.tensor_tensor(var[:, :], mu[:, s:s + 1], mu[:, s:s + 1], OP.mult)
        ve.scalar_tensor_tensor(var[:, :], x1sq[:, s:s + 1], 1.0 / HW,
                                var[:, :], OP.mult, OP.subtract)
        ve.tensor_scalar_add(var[:, :], var[:, :], float(EPS))
        # rstd = rsqrt(var) via bit trick + 2 Newton iterations
        ti = spool.tile([128, 1], mybir.dt.int32, name="ti", tag="ti")
        ve.tensor_scalar(ti[:, :], var[:, :].bitcast(mybir.dt.int32), 1,
                         None, OP.logical_shift_right)
        ve.tensor_tensor(ti[:, :], cs["magic"][:, 0:1], ti[:, :], OP.subtract)
        ry = spool.tile([128, 1], F32, name="ry", tag="ry")
        rt = spool.tile([128, 1], F32, name="rt", tag="rt")
        ve.tensor_copy(ry[:, :], ti[:, :].bitcast(F32))
        for _ in range(2):
            gp.tensor_tensor(rt[:, :], ry[:, :], ry[:, :], OP.mult)
            gp.tensor_tensor(rt[:, :], rt[:, :], var[:, :], OP.mult)
            gp.tensor_scalar(rt[:, :], rt[:, :], -0.5, 1.5, OP.mult, OP.add)
            gp.tensor_tensor(ry[:, :], ry[:, :], rt[:, :], OP.mult)
        gp.tensor_tensor(rstdw[:, s:s + 1], ry[:, :], cs["gnw"][:, 0:1],
                         OP.mult)

        # pooled x2 sums via edge algebra: A[p, tap]
        A = spool.tile([128, 9], F32, name="A", tag="A")
        Ap = A[:, :]
        gp.tensor_copy(Ap, xsum[:, s:s + 1].broadcast_to((128, 9)))
        out_r = custom_ap(Ap, 3, [[3, 2], [1, 3]])
        in_r = custom_ap(edges[:, :], 4 * s + 1, [[-1, 2], [0, 3]])
        gp.tensor_tensor(out_r, out_r, in_r, OP.subtract)
        out_c1 = custom_ap(Ap, 0, [[3, 3]])
        in_c1 = custom_ap(edges[:, :], 4 * s + 3, [[0, 3]])
        gp.tensor_tensor(out_c1, out_c1, in_c1, OP.subtract)
        out_c2 = custom_ap(Ap, 2, [[3, 3]])
        in_c2 = custom_ap(edges[:, :], 4 * s + 2, [[0, 3]])
        gp.tensor_tensor(out_c2, out_c2, in_c2, OP.subtract)
        out_k1 = custom_ap(Ap, 3, [[2, 2]])
        in_k1 = custom_ap(corners[:, :], 4 * s + 3, [[-1, 2]])
        gp.tensor_tensor(out_k1, out_k1, in_k1, OP.add)
        out_k2 = custom_ap(Ap, 6, [[2, 2]])
        in_k2 = custom_ap(corners[:, :], 4 * s + 1, [[-1, 2]])
        gp.tensor_tensor(out_k2, out_k2, in_k2, OP.add)
        Abf = spool.tile([128, 9], BF16, name="Abf", tag="Abf")
        sc.activation(Abf[:, :], A[:, :], AF.Copy)

        p2 = psmall.tile([128, 1], F32, name="p2", tag="ps_a")
        for t in range(9):
            te.matmul(p2[:, :], cs["W3sb"][:, 128 * t:128 * t + 128],
                      Abf[:, t:t + 1], start=(t == 0), stop=False)
        te.matmul(p2[:, :], cs["diagb3"][:, :], cs["c4096"][:, 0:1],
                  start=False, stop=True)
        m = spool.tile([128, 1], F32, name="m", tag="m")
        sc.activation(m[:, :], p2[:, :], AF.Copy, scale=1.0 / HW)
        # exp(m) by degree-4 Taylor (|m| << 1): e = 1+m(1+m(1/2+m(1/6+m/24)))
        e4 = spool.tile([128, 1], F32, name="e4", tag="e4")
        gp.tensor_scalar(e4[:, :], m[:, :], 1.0 / 24, 1.0 / 6, OP.mult,
                         OP.add)
        gp.tensor_tensor(e4[:, :], e4[:, :], m[:, :], OP.mult)
        gp.tensor_scalar_add(e4[:, :], e4[:, :], 0.5)
        gp.tensor_tensor(e4[:, :], e4[:, :], m[:, :], OP.mult)
        gp.tensor_scalar_add(e4[:, :], e4[:, :], 1.0)
        gp.tensor_tensor(e4[:, :], e4[:, :], m[:, :], OP.mult)
        gp.tensor_scalar_add(e4[:, :], e4[:, :], 1.0)
        ebf = spool.tile([128, 1], BF16, name="ebf", tag="ebf")
        sc.activation(ebf[:, :], e4[:, :], AF.Copy)
        s8 = psmall.tile([8, 1], F32, name="s8", tag="ps_b", bufs=1)
        te.matmul(s8[:, :], cs["maskblk"][:, :], ebf[:, :], start=True,
                  stop=True)
        s8bf = spool.tile([8, 1], BF16, name="s8bf", tag="s8bf")
        sc.activation(s8bf[:, :], s8[:, :], AF.Copy)
        rs8 = psmall.tile([128, 1], F32, name="rs8", tag="ps_b", bufs=1)
        te.matmul(rs8[:, :], cs["maskT"][:, :], s8bf[:, :], start=True,
                  stop=True)
        rec = spool.tile([128, 1], F32, name="rec", tag="rec")
        ve.reciprocal(rec[:, :], rs8[:, :])
        a21 = spool.tile([128, 1], F32, name="a21", tag="a21")
        gp.tensor_tensor(a21[:, :], e4[:, :], rec[:, :], OP.mult)
        gp.tensor_tensor(b21[:, s:s + 1], a21[:, :], rstdw[:, s:s + 1],
                         OP.mult)
        kv = spool.tile([128, 1], F32, name="kv", tag="kv")
        gp.tensor_tensor(kv[:, :], b21[:, s:s + 1], mu[:, s:s + 1], OP.mult)
        k2bf = spool.tile([128, 1], BF16, name="k2bf", tag="k2bf")
        ve.scalar_tensor_tensor(k2bf[:, :], a21[:, :], cs["gnb"][:, 0:1],
                                kv[:, :], OP.mult, OP.subtract)
        kps = psmall.tile([8, 1], F32, name="kps", tag="ps_a")
        te.matmul(kps[:, :], cs["maskblk"][:, :], k2bf[:, :], start=True,
                  stop=True)
        sc.activation(biaswv[:, s:s + 1], kps[:, :], AF.Copy,
                      bias=float(mean_b3_f))
        b21blk = spool.tile([128, 8], BF16, name="b21blk", tag="b21blk",
                            bufs=3)
        ve.tensor_tensor(b21blk[:, :], cs["maskblk"][:, :],
                         b21[:, s:s + 1].broadcast_to((128, 8)), OP.mult)

        # wv = mean_c(conv) + sum_c b21 x1 ; sigmoid with per-group bias
        sig = bpool.tile([8, HW], BF16, name="sig", tag="scratch", bufs=3)
        for half in range(2):
            wv = pwv.tile([128, 512], F32, name="wv", tag="wv")
            for t, (dy, dx) in enumerate(TAPS):
                for q in range(4):
                    hc = 4 * half + q
                    h0 = hc * 8
                    i0 = max(0, -(h0 + dy))
                    i1 = min(8, 64 - h0 - dy)
                    rhs = xp3[:, h0 + i0 + dy:h0 + i1 + dy,
                              2 + dx:2 + dx + W]
                    te.matmul(wv[32 * q:32 * q + 8, i0 * 64:i1 * 64],
                              cs["w3bar"][:, 8 * t:8 * t + 8], rhs,
                              start=(t == 0), stop=False,
                              tile_position=(0, 32 * q))
            for q in range(4):
                hc = 4 * half + q
                te.matmul(wv[32 * q:32 * q + 8, :], b21blk[:, :],
                          x1[:, 512 * hc:512 * hc + 512], start=False,
                          stop=True, tile_position=(0, 32 * q))
            for q in range(4):
                hc = 4 * half + q
                sc.activation(sig[:, 512 * hc:512 * hc + 512],
                              wv[32 * q:32 * q + 8, :],
                              AF.Sigmoid, bias=biaswv[:, s:s + 1])
        fused = fpool.tile([128, HW], F32, name="fused", tag="fused")
        fuseds.append(fused)
        fparts = spool.tile([128, 8], F32, name="fparts", tag="fparts")
        for hc in range(8):
            repq = prep.tile([128, 512], F32, name="repq", tag="repq")
            te.matmul(repq[:, :], cs["maskT"][:, :],
                      sig[:, 512 * hc:512 * hc + 512], start=True, stop=True)
            ve.scalar_tensor_tensor(
                fused[:, 512 * hc:512 * hc + 512].rearrange(
                    "p (h w) -> p h w", w=W),
                repq[:, :].rearrange("p (h w) -> p h w", w=W),
                scb[:, s:s + 1], xpi3[:, 8 * hc:8 * hc + 8, :],
                OP.add, OP.mult, accum_out=fparts[:, hc:hc + 1])
        ve.tensor_reduce(fsum[:, s:s + 1], fparts[:, :], AX.X, OP.add)

    # ---------------- global-attn SE over fused ---------------------------
    fsumbf = spool.tile([128, 4], BF16, name="fsumbf")
    sc.activation(fsumbf[:, :], fsum[:, :], AF.Copy)
    h2 = psmall.tile([32, 1], F32, name="h2", tag="ps_a")
    for s in range(NSLAB):
        te.matmul(h2[:, :], cs["ga1w"][:, 32 * s:32 * s + 32],
                  fsumbf[:, s:s + 1], start=(s == 0), stop=(s == 3))
    hid2 = spool.tile([32, 1], BF16, name="hid2", tag="hid")
    sc.activation(hid2[:, :], h2[:, :], AF.Relu, bias=cs["gab1"][:, 0:1])
    gps2 = psmall.tile([128, 4], F32, name="gps2", tag="ps_b", bufs=1)
    for s in range(NSLAB):
        te.matmul(gps2[:, s:s + 1], cs["ga2w"][:, 128 * s:128 * s + 128],
                  hid2[:, :], start=True, stop=True)
    for s in range(NSLAB):
        sc.activation(gga[:, s:s + 1], gps2[:, s:s + 1], AF.Sigmoid,
                      bias=cs["gab2"][:, s:s + 1])

    # ---------------- final gate + store -----------------------------------
    for s in range(NSLAB):
        fused = fuseds[s]
        sc.activation(fused[:, :], fused[:, :], AF.Copy,
                      scale=gga[:, s:s + 1])
        dst = y_d.ap()[b, s * 128:(s + 1) * 128, :, :]
        dst = bass.AP(dst.tensor, dst.offset, [list(dst.ap[0]), [1, HW]])
        nc.sync.dma_start(dst, fused[:, :])


def _ensure_ntff_hook():
    """run_bass_kernel_spmd(trace=True) under axon needs
    antenv.axon_hooks, which this image's antenv lacks. Shim it and
    register the ctypes-based NTFF hook from the boot package."""
    import types
    try:
        from antenv import axon_hooks  # noqa: F401
        return
    except ImportError:
        pass
    try:
        import antenv
        from trn_agent_boot.trn_boot import _ntff_profile_via_ctypes
        hooks = types.ModuleType("antenv.axon_hooks")
        _h = [None]
        hooks.set_axon_ntff_profile_hook = lambda h: _h.__setitem__(0, h)
        hooks.get_axon_ntff_profile_hook = lambda: _h[0]
        sys.modules["antenv.axon_hooks"] = hooks
        antenv.axon_hooks = hooks
        hooks.set_axon_ntff_profile_hook(
            _ntff_profile_via_ctypes("/opt/axon/libaxon_pjrt.so"))
    except Exception as e:  # profiling is best-effort
        print(f"ntff hook setup failed: {e}")


_CACHE = {}


def _get_program(consts, gamma_f, mean_b3_f):
    key = (float(gamma_f), float(mean_b3_f),
           tuple(sorted((k, v.tobytes()[:64].hex() if v.size > 16 else
                         v.tobytes().hex()) for k, v in consts.items())))
    key = hash(key)
    if key not in _CACHE:
        def _dt(v):
            if v.dtype == ml_dtypes.bfloat16:
                return BF16
            if v.dtype == np.int32:
                return mybir.dt.int32
            return F32
        const_shapes = {k: (v.shape, _dt(v)) for k, v in consts.items()}
        _CACHE[key] = build_program(gamma_f, mean_b3_f, const_shapes)
    return _CACHE[key]


def kernel(x, w1, b1, w3, b3, gn_w, gn_b, cg_w1, cg_b1, cg_w2, cg_b2,
           ga_w1, ga_b1, ga_w2, ga_b2, gamma, _return_timing=None):
    args = [np.asarray(a) for a in
            (x, w1, b1, w3, b3, gn_w, gn_b, cg_w1, cg_b1, cg_w2, cg_b2,
             ga_w1, ga_b1, ga_w2, ga_b2, gamma)]
    (x, w1, b1, w3, b3, gn_w, gn_b, cg_w1, cg_b1, cg_w2, cg_b2,
     ga_w1, ga_b1, ga_w2, ga_b2, gamma) = args
    consts = build_consts(w1, b1, w3, b3, gn_w, gn_b, cg_w1, cg_b1, cg_w2,
                          cg_b2, ga_w1, ga_b1, ga_w2, ga_b2, gamma)
    gamma_f = float(np.asarray(gamma).reshape(-1)[0])
    mean_b3_f = float(np.mean(b3))
    nc = _get_program(consts, gamma_f, mean_b3_f)

    in_maps = []
    for core in range(NCORES):
        m = {"x": _f32(x[core * BPC:(core + 1) * BPC])}
        m.update(consts)
        in_maps.append(m)
    trace = bool(_return_timing is not None)
    if trace:
        _ensure_ntff_hook()
    res = run_bass_kernel_spmd(nc, in_maps, core_ids=list(range(NCORES)),
                               trace=trace)
    if _return_timing is not None:
        _return_timing.update(dict(
            exec_time_ns=res.exec_time_ns,
            mean_exec_time_ns=res.mean_exec_time_ns,
        ))
    out = np.empty((B, C, H, W), np.float32)
    for core in range(NCORES):
        out[core * BPC:(core + 1) * BPC] = res.results[core]["y"]
    return out
